# revision 2
# baseline (speedup 1.0000x reference)
"""Trainium2 Bass kernel for nn_MinimalSSM: selective-scan SSM block, v2.

Single-NEFF design (no stage2): cores = batch(4) x d_half(2).  Each core
runs the recurrence for its 512 channels, then the cores of a batch
exchange y via a pairwise AllGather and each computes the out-projection
for its 512 OUTPUT columns with the full 1024-channel contraction in one
fp32 psum pass.

DVE-side changes vs v1:
  - persistent per-jb work tiles: the scan runs in-place (ht overwrites
    bt), carries are written by ACT directly into the work tile's two
    leading dummy columns, and the (0,1) a-dummy pattern is written once.
    This removes all per-chunk DVE copies.
  - the n-reduction tree ping-pongs inside the hct tile (in-place adds).
  - scans are issued per n-half (2 per jb-chunk) so the first scan starts
    after only 8 exps.
Startup: small-constant DMAs issue before the xt bulk so the first
proj/exp chain is not stuck behind 3.5 MB of queue.
"""

import numpy as np
import ml_dtypes

import concourse.bacc as bacc
import concourse.bass as bass
import concourse.tile as tile
from concourse import mybir
from concourse.bass_utils import run_bass_kernel_spmd
from contextlib import ExitStack

F32 = mybir.dt.float32
BF16 = mybir.dt.bfloat16
FP16 = mybir.dt.float16
AF = mybir.ActivationFunctionType
OP = mybir.AluOpType

B, T, D, N = 4, 2048, 1024, 16
DL = D // 2          # channels per core
NJB = DL // 128      # 4 local channel blocks
NKB = D // 128       # 8 contraction blocks for proj
NEB = DL // 128      # 4 output-column blocks (own half)
PC = 544             # proj columns per core: 512 delta + 16 B + 16 C
TC = 256             # time chunk
TCP = TC + 2
NCH = T // TC
NH = N // 2          # n-half for split scans

_cache = {}


def _pin_act_tables():
    import concourse.bacc as _bacc_mod
    from concourse.hw_specs import get_activation_tables as _orig

    def _only_nl_exp(arch):
        tabs = _orig(arch)
        return {k: (v if k == "natural_log_exp_and_others" else set())
                for k, v in tabs.items()}

    _bacc_mod.get_activation_tables = _only_nl_exp


_pin_act_tables()


def _build_stage1():
    nc = bacc.Bacc("TRN2", target_bir_lowering=False, debug=False, num_devices=8)
    xt = nc.dram_tensor("xt", [D, T], BF16, kind="ExternalInput")
    wx = nc.dram_tensor("wx", [D, PC], BF16, kind="ExternalInput")
    cst = nc.dram_tensor("cst", [128, 5 + NEB + NJB + NJB * N], F32,
                         kind="ExternalInput")
    wo = nc.dram_tensor("wo", [2 * D, DL], BF16, kind="ExternalInput")
    out = nc.dram_tensor("out", [DL, T], FP16, kind="ExternalOutput")
    bc_dram = nc.dram_tensor("bc_scratch", [NCH, 32, TC], BF16)

    # core pair: j = my d-half / out-col-half, peer holds the other one
    groups = [[2 * b, 2 * b + 1] for b in range(4)]

    with tile.TileContext(nc) as tc_ctx, ExitStack() as ctx:
        const = ctx.enter_context(tc_ctx.tile_pool(name="const", bufs=1))
        psum = ctx.enter_context(
            tc_ctx.tile_pool(name="psum", bufs=4, space="PSUM"))
        psumf = ctx.enter_context(
            tc_ctx.tile_pool(name="psumf", bufs=4, space="PSUM"))
        dpool = ctx.enter_context(tc_ctx.tile_pool(name="delta", bufs=6))
        dxpool = ctx.enter_context(tc_ctx.tile_pool(name="dx", bufs=6))
        bcpool = ctx.enter_context(tc_ctx.tile_pool(name="bc", bufs=2))
        reppool = ctx.enter_context(tc_ctx.tile_pool(name="rep", bufs=2))
        hcpool = ctx.enter_context(tc_ctx.tile_pool(name="hc", bufs=3))
        ybfpool = ctx.enter_context(tc_ctx.tile_pool(name="ybf", bufs=8))
        yppool = ctx.enter_context(tc_ctx.tile_pool(name="yp", bufs=16))
        popool = ctx.enter_context(tc_ctx.tile_pool(name="po", bufs=3))
        drampool = ctx.enter_context(
            tc_ctx.tile_pool(name="dramb", bufs=2, space="DRAM"))

        # one packed small-constant DMA, then merged wx / xt(chunk0) DMAs
        cst_sb = const.tile([128, 5 + NEB + NJB + NJB * N], F32, tag="cst")
        nc.sync.dma_start(cst_sb[:], cst[:])
        wxm = const.tile([128, NKB, PC], BF16, tag="wxm")
        nc.sync.dma_start(wxm[:, :, 4 * 128:],
                          wx[:, 4 * 128:].rearrange("(k p) c -> p k c", p=128))
        xtm = const.tile([128, NKB, T], BF16, tag="xtm")
        nc.sync.dma_start(xtm[:, :, 0:TC],
                          xt[:, 0:TC].rearrange("(k p) c -> p k c", p=128))
        nc.sync.dma_start(wxm[:, :, 0:4 * 128],
                          wx[:, 0:4 * 128].rearrange("(k p) c -> p k c", p=128))
        aneg_sb = cst_sb
        wom = const.tile([128, 2 * NKB, DL], BF16, tag="wom")

        # persistent a tiles: dummy (0,1) reset columns written once
        at_t = []
        for q in range(8):
            att = const.tile([128, NH, TCP], BF16, tag=f"att{q}")
            nc.vector.memset(att[:, :, 0:1], 0.0)
            nc.vector.memset(att[:, :, 1:2], 1.0)
            at_t.append(att)
        # persistent per-jb work tiles (b, then h in-place); carries are
        # ACT-written into cols 0:2 at chunk boundaries
        wk = []
        for jb in range(NJB):
            wkt = const.tile([128, N, TCP], BF16, tag=f"wk{jb}")
            nc.vector.memset(wkt[:, :, 0:2], 0.0)
            wk.append(wkt)

        def ship_y(y_list, plen, tag):
            """AllGather a y piece; return the 8 gathered kb-block tiles."""
            ybin_t = drampool.tile([len(y_list) * 128, plen], BF16,
                                   tag=f"ybin{tag}")
            ybout_t = drampool.tile([2, len(y_list) * 128, plen], BF16,
                                    tag=f"ybout{tag}")
            for q, yb in enumerate(y_list):
                nc.gpsimd.dma_start(ybin_t[q * 128:(q + 1) * 128, :], yb)
            nc.gpsimd.collective_compute(
                "AllGather", OP.bypass, replica_groups=groups,
                ins=[ybin_t[:]], outs=[ybout_t[:]])
            return ybout_t

        def y_fold(ybout_t, jb0p, npc, P, stop):
            """Read gathered blocks, accumulate into the held psums."""
            for sl_i in range(2):
                for q in range(npc):
                    kb = sl_i * 4 + jb0p + q
                    ypt = yppool.tile([128, TC], BF16, tag="yp")
                    nc.sync.dma_start(
                        ypt[:], ybout_t[sl_i, q * 128:(q + 1) * 128, :])
                    for eb in range(NEB):
                        nc.tensor.matmul(
                            P[eb][:, 0:TC],
                            wom[:, kb, eb * 128:(eb + 1) * 128],
                            ypt[:],
                            start=False,
                            stop=(stop and sl_i == 1 and q == npc - 1))

        def evac(P, t0):
            for eb in range(NEB):
                pot = popool.tile([128, TC], FP16, tag="po")
                nc.scalar.activation(pot[:], P[eb][:, 0:TC], AF.Identity,
                                     bias=cst_sb[:, 5 + eb:6 + eb])
                nc.sync.dma_start(out[eb * 128:(eb + 1) * 128, t0:t0 + TC],
                                  pot[:])

        pending = None   # (ybout_t, P, t0) of the previous chunk
        for ci in range(NCH):
            t0 = ci * TC
            last = ci == NCH - 1
            delta_tiles = []
            bct = None
            for m in (4, 0, 1, 2, 3):   # B/C first: unblocks the broadcasts
                mm = 128 if m < 4 else 32
                ps = psum.tile([mm, TC], F32, tag="ps")
                for kb in range(NKB):
                    nc.tensor.matmul(
                        ps[:],
                        wxm[:, kb, m * 128:m * 128 + mm],
                        xtm[:, kb, t0:t0 + TC],
                        start=(kb == 0), stop=(kb == NKB - 1))
                if m < 4:
                    et_ = dpool.tile([128, TC], F32, tag="etmp")
                    nc.scalar.activation(et_[:], ps[:], AF.Exp,
                                         bias=cst_sb[:, m:m + 1])
                    dt_ = dpool.tile([128, TC], BF16, tag="delta")
                    nc.scalar.activation(dt_[:], et_[:], AF.Ln, bias=1.0)
                    delta_tiles.append(dt_)
                    for h in range(2):
                        att = at_t[2 * m + h]
                        for n in range(NH):
                            gn = h * NH + n
                            nc.scalar.activation(
                                att[:, n, 2:], dt_[:], AF.Exp,
                                scale=cst_sb[:, 13 + m * N + gn:14 + m * N + gn])
                else:
                    bct = bcpool.tile([32, TC], BF16, tag="bc")
                    nc.scalar.activation(bct[:], ps[:], AF.Identity,
                                         bias=cst_sb[:32, 4:5])
                    # kick the broadcast round trip before the delta projs
                    nc.sync.dma_start(bc_dram[ci], bct[:])
                    brep = reppool.tile([128, N, TC], BF16, tag="brep")
                    crep = reppool.tile([128, N, TC], BF16, tag="crep")
                    nc.sync.dma_start(
                        brep[:], bc_dram[ci, 0:N, :].partition_broadcast(128))
                    nc.sync.dma_start(
                        crep[:],
                        bc_dram[ci, N:2 * N, :].partition_broadcast(128))
            if ci == 0:
                # bulk xt + wo ride the ACT hwdge queue, off the sync queue
                nc.scalar.dma_start(
                    xtm[:, :, TC:],
                    xt[:, TC:].rearrange("(k p) c -> p k c", p=128))
                nc.scalar.dma_start(
                    wom[:], wo[:].rearrange("(k p) c -> p k c", p=128))

            # previous chunk's y-contraction + evacuation
            if pending is not None:
                pybo, pP, pt0 = pending
                y_fold(pybo, 0, NJB, pP, stop=True)
                evac(pP, pt0)

            # this chunk's held out-psums; x @ (Dp-folded W_out) part
            P = []
            for eb in range(NEB):
                fps = psumf.tile([128, 512], F32, tag="fps")
                P.append(fps)

            for eb in range(NEB):
                for kb in range(NKB):
                    nc.tensor.matmul(
                        P[eb][:, 0:TC],
                        wom[:, NKB + kb, eb * 128:(eb + 1) * 128],
                        xtm[:, kb, t0:t0 + TC],
                        start=(kb == 0), stop=False)

            fin_y = []
            for jb in range(NJB):
                dt_ = delta_tiles[jb]
                w = wk[jb]
                dxt = dxpool.tile([128, TC], BF16, tag="dx")
                nc.vector.tensor_mul(dxt[:], dt_[:], xtm[:, jb, t0:t0 + TC])
                ats = [at_t[2 * jb + h] for h in range(2)]
                dx_b = dxt[:].unsqueeze(1).broadcast_to([128, N, TC])
                nc.vector.tensor_mul(w[:, :, 2:], dx_b, brep[:])
                for h in range(2):
                    sl = w[:, h * NH:(h + 1) * NH, :]
                    nc.vector.tensor_tensor_scan(
                        sl.rearrange("p n t -> p (n t)"),
                        ats[h][:].rearrange("p n t -> p (n t)"),
                        sl.rearrange("p n t -> p (n t)"),
                        0.0, op0=OP.mult, op1=OP.add)
                if not last:
                    nc.scalar.activation(w[:, :, 0:1], w[:, :, TCP - 1:TCP],
                                         AF.Identity)
                    nc.scalar.activation(w[:, :, 1:2], w[:, :, TCP - 1:TCP],
                                         AF.Copy, bias=0.0, scale=0.0)
                hct = hcpool.tile([128, N, TC], BF16, tag="hc")
                nc.vector.tensor_mul(hct[:], w[:, :, 2:], crep[:])
                nc.vector.tensor_add(hct[:, 0:8, :], hct[:, 0:8, :],
                                     hct[:, 8:16, :])
                nc.vector.tensor_add(hct[:, 0:4, :], hct[:, 0:4, :],
                                     hct[:, 4:8, :])
                nc.vector.tensor_add(hct[:, 0:2, :], hct[:, 0:2, :],
                                     hct[:, 2:4, :])
                ybt = ybfpool.tile([128, TC], BF16, tag="ybf")
                nc.vector.tensor_add(ybt[:], hct[:, 0, :], hct[:, 1, :])
                fin_y.append(ybt)
                if last and jb in (2, 3):
                    # pieces (jb0-2) and (jb3): big CC launches early, only
                    # the small one sits in the tail
                    npc = len(fin_y)
                    jb0p = jb - npc + 1
                    ybo = ship_y([yb[:] for yb in fin_y], TC, f"p{npc}")
                    fin_y = []
                    y_fold(ybo, jb0p, npc, P, stop=(jb == 3))

            if not last:
                ybo = ship_y([yb[:] for yb in fin_y], TC, "full")
                pending = (ybo, P, t0)
            else:
                evac(P, t0)
    nc.compile()
    return nc


def _stage1_inputs(x, A_log, Dp, W_xproj, b_xproj, W_out, b_out):
    bf = ml_dtypes.bfloat16
    in_maps = []
    for c in range(8):
        b, j = c // 2, c % 2
        lo, hi = j * DL, (j + 1) * DL
        order = np.concatenate(
            [np.arange(lo, hi), np.arange(0, lo), np.arange(hi, D)])
        cols = np.concatenate([np.arange(lo, hi), np.arange(D, D + 2 * N)])
        xt_full = np.ascontiguousarray(x[b].T[order]).astype(bf)
        wxc = np.ascontiguousarray(W_xproj[order][:, cols]).astype(bf)
        bx_pad = np.zeros(5 * 128, np.float32)
        bx_pad[:PC] = b_xproj[cols]
        bx_arr = bx_pad.reshape(5, 128).T
        alog_l = -np.exp(A_log[lo:hi].reshape(NJB, 128, N)
                         .transpose(1, 0, 2).reshape(128, NJB * N))
        dp_l = Dp[lo:hi].reshape(NJB, 128).T
        # out-proj: contraction over ALL channels in AllGather slot order
        # (slot0 = core (b,0) channels 0..511, slot1 = core (b,1) 512..1023);
        # output columns = this core's half.
        ocols = np.arange(lo, hi)
        w2 = (Dp[:, None] * W_out)[order][:, ocols]
        wo_l = np.ascontiguousarray(
            np.concatenate([W_out[:, ocols], w2], axis=0)).astype(bf)
        bo_l = b_out[ocols].reshape(NEB, 128).T.astype(np.float32)
        cst_l = np.ascontiguousarray(np.concatenate(
            [bx_arr, bo_l, dp_l, alog_l], axis=1).astype(np.float32))
        in_maps.append({
            "xt": xt_full, "wx": wxc, "cst": cst_l, "wo": wo_l,
        })
    return in_maps


def kernel(x, A_log, Dp, W_xproj, b_xproj, W_out, b_out, _trace=False):
    x = np.asarray(x, np.float32)
    A_log = np.asarray(A_log, np.float32)
    Dp = np.asarray(Dp, np.float32)
    W_xproj = np.asarray(W_xproj, np.float32)
    b_xproj = np.asarray(b_xproj, np.float32)
    W_out = np.asarray(W_out, np.float32)
    b_out = np.asarray(b_out, np.float32)

    if "s1" not in _cache:
        _cache["s1"] = _build_stage1()

    in1 = _stage1_inputs(x, A_log, Dp, W_xproj, b_xproj, W_out, b_out)
    kw = dict(trace=True, trace_cores=list(range(8))) if _trace else {}
    res1 = run_bass_kernel_spmd(_cache["s1"], in1, core_ids=list(range(8)), **kw)

    outs = []
    for b in range(4):
        o0 = np.asarray(res1.results[2 * b]["out"], np.float32)     # cols 0..511
        o1 = np.asarray(res1.results[2 * b + 1]["out"], np.float32)  # cols 512..1023
        outs.append(np.concatenate([o0, o1], axis=0).T)
    out = np.stack(outs).astype(np.float32)
    if _trace:
        return out, (res1,)
    return out


# revision 3
# speedup vs baseline: 1.0187x; 1.0187x over previous
"""Trainium2 Bass kernel for nn_MinimalSSM: selective-scan SSM block, v2.

Single-NEFF design (no stage2): cores = batch(4) x d_half(2).  Each core
runs the recurrence for its 512 channels, then the cores of a batch
exchange y via a pairwise AllGather and each computes the out-projection
for its 512 OUTPUT columns with the full 1024-channel contraction in one
fp32 psum pass.

DVE-side changes vs v1:
  - persistent per-jb work tiles: the scan runs in-place (ht overwrites
    bt), carries are written by ACT directly into the work tile's two
    leading dummy columns, and the (0,1) a-dummy pattern is written once.
    This removes all per-chunk DVE copies.
  - the n-reduction tree ping-pongs inside the hct tile (in-place adds).
  - scans are issued per n-half (2 per jb-chunk) so the first scan starts
    after only 8 exps.
Startup: small-constant DMAs issue before the xt bulk so the first
proj/exp chain is not stuck behind 3.5 MB of queue.
"""

import numpy as np
import ml_dtypes

import concourse.bacc as bacc
import concourse.bass as bass
import concourse.tile as tile
from concourse import mybir
from concourse.bass_utils import run_bass_kernel_spmd
from contextlib import ExitStack

F32 = mybir.dt.float32
BF16 = mybir.dt.bfloat16
FP16 = mybir.dt.float16
AF = mybir.ActivationFunctionType
OP = mybir.AluOpType

B, T, D, N = 4, 2048, 1024, 16
DL = D // 2          # channels per core
NJB = DL // 128      # 4 local channel blocks
NKB = D // 128       # 8 contraction blocks for proj
NEB = DL // 128      # 4 output-column blocks (own half)
PC = 544             # proj columns per core: 512 delta + 16 B + 16 C
TC = 256             # time chunk
TCP = TC + 2
NCH = T // TC
NH = N // 2          # n-half for split scans

_cache = {}


def _pin_act_tables():
    import concourse.bacc as _bacc_mod
    from concourse.hw_specs import get_activation_tables as _orig

    def _only_nl_exp(arch):
        tabs = _orig(arch)
        return {k: (v if k == "natural_log_exp_and_others" else set())
                for k, v in tabs.items()}

    _bacc_mod.get_activation_tables = _only_nl_exp


_pin_act_tables()


def _build_stage1():
    nc = bacc.Bacc("TRN2", target_bir_lowering=False, debug=False, num_devices=8)
    xt = nc.dram_tensor("xt", [D, T], BF16, kind="ExternalInput")
    wx = nc.dram_tensor("wx", [D, PC], BF16, kind="ExternalInput")
    cst = nc.dram_tensor("cst", [128, 5 + NEB + NJB + NJB * N], F32,
                         kind="ExternalInput")
    wo = nc.dram_tensor("wo", [2 * D, DL], BF16, kind="ExternalInput")
    out = nc.dram_tensor("out", [DL, T], FP16, kind="ExternalOutput")
    bc_dram = nc.dram_tensor("bc_scratch", [NCH, 32, TC], BF16)

    # core pair: j = my d-half / out-col-half, peer holds the other one
    groups = [[2 * b, 2 * b + 1] for b in range(4)]

    with tile.TileContext(nc) as tc_ctx, ExitStack() as ctx:
        const = ctx.enter_context(tc_ctx.tile_pool(name="const", bufs=1))
        psum = ctx.enter_context(
            tc_ctx.tile_pool(name="psum", bufs=4, space="PSUM"))
        psumf = ctx.enter_context(
            tc_ctx.tile_pool(name="psumf", bufs=4, space="PSUM"))
        dpool = ctx.enter_context(tc_ctx.tile_pool(name="delta", bufs=6))
        dxpool = ctx.enter_context(tc_ctx.tile_pool(name="dx", bufs=6))
        bcpool = ctx.enter_context(tc_ctx.tile_pool(name="bc", bufs=2))
        reppool = ctx.enter_context(tc_ctx.tile_pool(name="rep", bufs=2))
        hcpool = ctx.enter_context(tc_ctx.tile_pool(name="hc", bufs=3))
        ybfpool = ctx.enter_context(tc_ctx.tile_pool(name="ybf", bufs=8))
        yppool = ctx.enter_context(tc_ctx.tile_pool(name="yp", bufs=16))
        popool = ctx.enter_context(tc_ctx.tile_pool(name="po", bufs=3))
        drampool = ctx.enter_context(
            tc_ctx.tile_pool(name="dramb", bufs=2, space="DRAM"))

        # one packed small-constant DMA, then merged wx / xt(chunk0) DMAs
        cst_sb = const.tile([128, 5 + NEB + NJB + NJB * N], F32, tag="cst")
        nc.sync.dma_start(cst_sb[:], cst[:])
        wxm = const.tile([128, NKB, PC], BF16, tag="wxm")
        nc.sync.dma_start(wxm[:, :, 4 * 128:],
                          wx[:, 4 * 128:].rearrange("(k p) c -> p k c", p=128))
        xtm = const.tile([128, NKB, T], BF16, tag="xtm")
        nc.sync.dma_start(xtm[:, :, 0:TC],
                          xt[:, 0:TC].rearrange("(k p) c -> p k c", p=128))
        nc.sync.dma_start(wxm[:, :, 0:4 * 128],
                          wx[:, 0:4 * 128].rearrange("(k p) c -> p k c", p=128))
        aneg_sb = cst_sb
        wom = const.tile([128, 2 * NKB, DL], BF16, tag="wom")

        # persistent a tiles: dummy (0,1) reset columns written once
        at_t = []
        for q in range(8):
            att = const.tile([128, NH, TCP], BF16, tag=f"att{q}")
            nc.vector.memset(att[:, :, 0:1], 0.0)
            nc.vector.memset(att[:, :, 1:2], 1.0)
            at_t.append(att)
        # persistent per-jb work tiles (b, then h in-place); carries are
        # ACT-written into cols 0:2 at chunk boundaries
        wk = []
        for jb in range(NJB):
            wkt = const.tile([128, N, TCP], BF16, tag=f"wk{jb}")
            nc.vector.memset(wkt[:, :, 0:2], 0.0)
            wk.append(wkt)

        def ship_y(y_list, plen, tag):
            """AllGather a y piece; return the 8 gathered kb-block tiles."""
            ybin_t = drampool.tile([len(y_list) * 128, plen], BF16,
                                   tag=f"ybin{tag}")
            ybout_t = drampool.tile([2, len(y_list) * 128, plen], BF16,
                                    tag=f"ybout{tag}")
            for q, yb in enumerate(y_list):
                nc.gpsimd.dma_start(ybin_t[q * 128:(q + 1) * 128, :], yb)
            nc.gpsimd.collective_compute(
                "AllGather", OP.bypass, replica_groups=groups,
                ins=[ybin_t[:]], outs=[ybout_t[:]])
            return ybout_t

        def y_fold(ybout_t, jb0p, npc, P, stop):
            """Read gathered blocks, accumulate into the held psums."""
            for sl_i in range(2):
                for q in range(npc):
                    kb = sl_i * 4 + jb0p + q
                    ypt = yppool.tile([128, TC], BF16, tag="yp")
                    nc.sync.dma_start(
                        ypt[:], ybout_t[sl_i, q * 128:(q + 1) * 128, :])
                    for eb in range(NEB):
                        nc.tensor.matmul(
                            P[eb][:, 0:TC],
                            wom[:, kb, eb * 128:(eb + 1) * 128],
                            ypt[:],
                            start=False,
                            stop=(stop and sl_i == 1 and q == npc - 1))

        def evac(P, t0):
            for eb in range(NEB):
                pot = popool.tile([128, TC], FP16, tag="po")
                nc.scalar.activation(pot[:], P[eb][:, 0:TC], AF.Identity,
                                     bias=cst_sb[:, 5 + eb:6 + eb])
                nc.sync.dma_start(out[eb * 128:(eb + 1) * 128, t0:t0 + TC],
                                  pot[:])

        pending = None   # (ybout_t, P, t0) of the previous chunk
        for ci in range(NCH):
            t0 = ci * TC
            last = ci == NCH - 1
            delta_tiles = []
            bct = None
            for m in (4, 0, 1, 2, 3):   # B/C first: unblocks the broadcasts
                mm = 128 if m < 4 else 32
                ps = psum.tile([mm, TC], F32, tag="ps")
                for kb in range(NKB):
                    nc.tensor.matmul(
                        ps[:],
                        wxm[:, kb, m * 128:m * 128 + mm],
                        xtm[:, kb, t0:t0 + TC],
                        start=(kb == 0), stop=(kb == NKB - 1))
                if m < 4:
                    et_ = dpool.tile([128, TC], F32, tag="etmp")
                    nc.scalar.activation(et_[:], ps[:], AF.Exp,
                                         bias=cst_sb[:, m:m + 1])
                    dt_ = dpool.tile([128, TC], BF16, tag="delta")
                    nc.scalar.activation(dt_[:], et_[:], AF.Ln, bias=1.0)
                    delta_tiles.append(dt_)
                    for h in range(2):
                        att = at_t[2 * m + h]
                        for n in range(NH):
                            gn = h * NH + n
                            nc.scalar.activation(
                                att[:, n, 2:], dt_[:], AF.Exp,
                                scale=cst_sb[:, 13 + m * N + gn:14 + m * N + gn])
                else:
                    bct = bcpool.tile([32, TC], BF16, tag="bc")
                    nc.scalar.activation(bct[:], ps[:], AF.Identity,
                                         bias=cst_sb[:32, 4:5])
                    # kick the broadcast round trip before the delta projs
                    nc.sync.dma_start(bc_dram[ci], bct[:])
                    bcrep = reppool.tile([128, 2 * N, TC], BF16, tag="bcrep")
                    nc.sync.dma_start(
                        bcrep[:], bc_dram[ci].partition_broadcast(128))
                    brep = bcrep[:, 0:N, :]
                    crep = bcrep[:, N:2 * N, :]
            if ci == 0:
                # bulk xt + wo ride the ACT hwdge queue, off the sync queue
                nc.scalar.dma_start(
                    xtm[:, :, TC:],
                    xt[:, TC:].rearrange("(k p) c -> p k c", p=128))
                nc.scalar.dma_start(
                    wom[:], wo[:].rearrange("(k p) c -> p k c", p=128))

            # previous chunk's y-contraction + evacuation
            if pending is not None:
                pybo, pP, pt0 = pending
                y_fold(pybo, 0, NJB, pP, stop=True)
                evac(pP, pt0)

            # this chunk's held out-psums; x @ (Dp-folded W_out) part
            P = []
            for eb in range(NEB):
                fps = psumf.tile([128, 512], F32, tag="fps")
                P.append(fps)

            for eb in range(NEB):
                for kb in range(NKB):
                    nc.tensor.matmul(
                        P[eb][:, 0:TC],
                        wom[:, NKB + kb, eb * 128:(eb + 1) * 128],
                        xtm[:, kb, t0:t0 + TC],
                        start=(kb == 0), stop=False)

            fin_y = []
            for jb in range(NJB):
                dt_ = delta_tiles[jb]
                w = wk[jb]
                dxt = dxpool.tile([128, TC], BF16, tag="dx")
                nc.vector.tensor_mul(dxt[:], dt_[:], xtm[:, jb, t0:t0 + TC])
                ats = [at_t[2 * jb + h] for h in range(2)]
                dx_b = dxt[:].unsqueeze(1).broadcast_to([128, N, TC])
                nc.vector.tensor_mul(w[:, :, 2:], dx_b, brep)
                for h in range(2):
                    sl = w[:, h * NH:(h + 1) * NH, :]
                    nc.vector.tensor_tensor_scan(
                        sl.rearrange("p n t -> p (n t)"),
                        ats[h][:].rearrange("p n t -> p (n t)"),
                        sl.rearrange("p n t -> p (n t)"),
                        0.0, op0=OP.mult, op1=OP.add)
                if not last:
                    nc.scalar.activation(w[:, :, 0:1], w[:, :, TCP - 1:TCP],
                                         AF.Identity)
                    nc.scalar.activation(w[:, :, 1:2], w[:, :, TCP - 1:TCP],
                                         AF.Copy, bias=0.0, scale=0.0)
                hct = hcpool.tile([128, N, TC], BF16, tag="hc")
                nc.vector.tensor_mul(hct[:], w[:, :, 2:], crep)
                nc.vector.tensor_add(hct[:, 0:8, :], hct[:, 0:8, :],
                                     hct[:, 8:16, :])
                nc.vector.tensor_add(hct[:, 0:4, :], hct[:, 0:4, :],
                                     hct[:, 4:8, :])
                nc.vector.tensor_add(hct[:, 0:2, :], hct[:, 0:2, :],
                                     hct[:, 2:4, :])
                ybt = ybfpool.tile([128, TC], BF16, tag="ybf")
                nc.vector.tensor_add(ybt[:], hct[:, 0, :], hct[:, 1, :])
                fin_y.append(ybt)
                if last and jb in (2, 3):
                    # pieces (jb0-2) and (jb3): big CC launches early, only
                    # the small one sits in the tail
                    npc = len(fin_y)
                    jb0p = jb - npc + 1
                    ybo = ship_y([yb[:] for yb in fin_y], TC, f"p{npc}")
                    fin_y = []
                    y_fold(ybo, jb0p, npc, P, stop=(jb == 3))

            if not last:
                ybo = ship_y([yb[:] for yb in fin_y], TC, "full")
                pending = (ybo, P, t0)
            else:
                evac(P, t0)
    nc.compile()
    return nc


def _stage1_inputs(x, A_log, Dp, W_xproj, b_xproj, W_out, b_out):
    bf = ml_dtypes.bfloat16
    in_maps = []
    for c in range(8):
        b, j = c // 2, c % 2
        lo, hi = j * DL, (j + 1) * DL
        order = np.concatenate(
            [np.arange(lo, hi), np.arange(0, lo), np.arange(hi, D)])
        cols = np.concatenate([np.arange(lo, hi), np.arange(D, D + 2 * N)])
        xt_full = np.ascontiguousarray(x[b].T[order]).astype(bf)
        wxc = np.ascontiguousarray(W_xproj[order][:, cols]).astype(bf)
        bx_pad = np.zeros(5 * 128, np.float32)
        bx_pad[:PC] = b_xproj[cols]
        bx_arr = bx_pad.reshape(5, 128).T
        alog_l = -np.exp(A_log[lo:hi].reshape(NJB, 128, N)
                         .transpose(1, 0, 2).reshape(128, NJB * N))
        dp_l = Dp[lo:hi].reshape(NJB, 128).T
        # out-proj: contraction over ALL channels in AllGather slot order
        # (slot0 = core (b,0) channels 0..511, slot1 = core (b,1) 512..1023);
        # output columns = this core's half.
        ocols = np.arange(lo, hi)
        w2 = (Dp[:, None] * W_out)[order][:, ocols]
        wo_l = np.ascontiguousarray(
            np.concatenate([W_out[:, ocols], w2], axis=0)).astype(bf)
        bo_l = b_out[ocols].reshape(NEB, 128).T.astype(np.float32)
        cst_l = np.ascontiguousarray(np.concatenate(
            [bx_arr, bo_l, dp_l, alog_l], axis=1).astype(np.float32))
        in_maps.append({
            "xt": xt_full, "wx": wxc, "cst": cst_l, "wo": wo_l,
        })
    return in_maps


def kernel(x, A_log, Dp, W_xproj, b_xproj, W_out, b_out, _trace=False):
    x = np.asarray(x, np.float32)
    A_log = np.asarray(A_log, np.float32)
    Dp = np.asarray(Dp, np.float32)
    W_xproj = np.asarray(W_xproj, np.float32)
    b_xproj = np.asarray(b_xproj, np.float32)
    W_out = np.asarray(W_out, np.float32)
    b_out = np.asarray(b_out, np.float32)

    if "s1" not in _cache:
        _cache["s1"] = _build_stage1()

    in1 = _stage1_inputs(x, A_log, Dp, W_xproj, b_xproj, W_out, b_out)
    kw = dict(trace=True, trace_cores=list(range(8))) if _trace else {}
    res1 = run_bass_kernel_spmd(_cache["s1"], in1, core_ids=list(range(8)), **kw)

    outs = []
    for b in range(4):
        o0 = np.asarray(res1.results[2 * b]["out"], np.float32)     # cols 0..511
        o1 = np.asarray(res1.results[2 * b + 1]["out"], np.float32)  # cols 512..1023
        outs.append(np.concatenate([o0, o1], axis=0).T)
    out = np.stack(outs).astype(np.float32)
    if _trace:
        return out, (res1,)
    return out
